# revision 19
# baseline (speedup 1.0000x reference)
"""Trainium2 Bass kernel for EquidistantDiscreteContinuousConv3d.

Math: out = conv3d(x, einsum('ogk,kzyx->ogzyx', weight, psi_local), stride 2,
pad 2) + bias, with x [2,8,128,128,128] -> out [2,16,64,64,64].

The dense 5^3 kernel only has taps within Euclidean radius 2 (33 of 125
offsets are nonzero). Sharding: 8 cores = batch(2) x y-quarters(4); each core
computes out[b, :, :, 16q:16q+16] from an overlapping, zero-padded input
slab. No collectives -- halos materialize as overlapping host-side slices.

Device mapping: the tensor engine contracts K = (z_window(16) x ic(8)) = 128
partitions, with M = (oz_sub x oc(16)) packed into a block-banded weight
matrix (band encodes the dz taps), looped over the 13 (dy, dx) stencil taps
that accumulate in PSUM. Giving each core the FULL z extent (64 oz) makes
the 6-oz-per-16-plane-window blocking waste only boundary slots. Four lead
units cover 7 oz each via a shifted window [2*oz0-1, +16) plus a one-matmul
"fixup" (the single missing (oz0, dz=-2, dy=0, dx=0) combo) that contracts
8 K-rows of the PREVIOUS unit's tile; the remaining six units are plain
6-oz windows. 10 units x 2 oy-halves = 20 PSUM groups, 266 matmuls of
N=512 vs 312 for a uniform-6 z-sharded layout.

rhs slices come from a phase-decomposed (even/odd y and x, de-interleaved so
the innermost 64 x-positions are contiguous) view of the input tile. Each
unit arrives as two parallel half-DMAs (yo rows 0..9 on the SP queue, rows
10..17 on the ACT queue) so tile 0 lands fast; wtile and the 10 output DMAs
ride the otherwise-idle GPSIMD queue. Output is written bf16 (cast on the
psum->stage copy) and widened to f32 on the host.

Raw Bacc pipeline per core (static, fully unrolled; no TileContext):
  SP    : 10 paced input A-half DMAs + even-unit output DMAs, end sem clear
  ACT   : wtile DMA, 10 paced input B-half DMAs + odd-unit output DMAs
  PE    : short HAM warm-up, then 20 groups x (13|14) banded matmuls
  DVE   : 20 psum->stage bf16 copies (stage slot = unit%2)
"""

import os

import ml_dtypes
import numpy as np

BF16 = ml_dtypes.bfloat16

IC, OC = 8, 16
TAPS_XY = [
    (dy, dx) for dy in range(-2, 3) for dx in range(-2, 3) if dy * dy + dx * dx <= 4
]  # 13 taps
SUB_FREE = 36 * 132  # per-unit free size: (yo 18, yp 2, px 2, xe 66)
ROW = 2 * 2 * 66  # one yo row = (yp, px, xe) block of 264 elements
N_CORES = 8
NSLOT = 6
WARMUP = 112

# units: (oz0, noz, zp0, wkind, fixup) -- zp = z + 2 (host pad), window is
# zp [zp0, zp0+16). wkind 0: band rel plane = 2*ozs+dzi (6-oz); wkind 1:
# rel = 2*ozs+dzi-1 (7-oz, shifted window).
UNITS = [(7 * u, 7, 14 * u + 1, 1, u > 0) for u in range(4)] + [
    (28 + 6 * k, 6, 56 + 12 * k, 0, False) for k in range(6)
]
NU = len(UNITS)
NW = 29  # 13 (6-oz band) + 13 (7-oz) + fixup + 2 merged-corner matrices
MERGED = ((0, 2), (-2, 0))  # corner-tap pair computed by one matmul
REG_TAPS = [t for t in TAPS_XY if t not in MERGED]  # 11 regular taps
CFREE = 18 * 66  # corner tile free size (ay 18, xv 66)
NSLOTC = 3
ZP = 132  # padded z planes on host (z = zp - 2)

_MODULE = None
LAST_RESULT = None  # BassKernelResults of the most recent run (for test harness)


def _build_module():
    from contextlib import ExitStack

    import concourse.bacc as bacc
    import concourse.mybir as mybir

    f32 = mybir.dt.float32
    bf16 = mybir.dt.bfloat16

    nc = bacc.Bacc()
    x_in = nc.dram_tensor("xc", [NU, 128, SUB_FREE], bf16, kind="ExternalInput")
    c_in = nc.dram_tensor("cc", [NU, 128, CFREE], bf16, kind="ExternalInput")
    w_in = nc.dram_tensor("wc", [128, NW * 128], bf16, kind="ExternalInput")
    out = nc.dram_tensor("out", [64, 16, 2, 8, 64], bf16, kind="ExternalOutput")

    with ExitStack() as ctx:
        wsem = ctx.enter_context(nc.semaphore("wsem"))
        ws2 = ctx.enter_context(nc.semaphore("ws2"))
        a0s = ctx.enter_context(nc.semaphore("a0s"))
        cq = [ctx.enter_context(nc.semaphore(f"cq{i}")) for i in range(2)]
        xsA = [ctx.enter_context(nc.semaphore(f"xsemA{i}")) for i in range(2)]
        xsB = [ctx.enter_context(nc.semaphore(f"xsemB{i}")) for i in range(2)]
        pesem = ctx.enter_context(nc.semaphore("pesem"))
        dvsem = ctx.enter_context(nc.semaphore("dvsem"))
        oss = [ctx.enter_context(nc.semaphore(f"osem{i}")) for i in range(2)]
        wtile = ctx.enter_context(nc.sbuf_tensor("wtile", [128, NW * 128], bf16))
        xts = [
            ctx.enter_context(nc.sbuf_tensor(f"xt{i}", [128, SUB_FREE], bf16))
            for i in range(NSLOT)
        ]
        cts = [
            ctx.enter_context(nc.sbuf_tensor(f"ct{i}", [128, CFREE], bf16))
            for i in range(NSLOTC)
        ]
        stgs = [
            ctx.enter_context(nc.sbuf_tensor(f"stg{i}", [128, 2 * 512], bf16))
            for i in range(2)
        ]
        pss = [
            ctx.enter_context(nc.psum_tensor(f"ps{i}", [128, 512], f32))
            for i in range(8)
        ]
        x5s = [
            t[:].rearrange("p (a b d c) -> p a b d c", a=18, b=2, d=2, c=66)
            for t in xts
        ]
        c5s = [
            t[:].rearrange("p (a c) -> p a c", a=18, c=66) for t in cts
        ]

        def cdma(eng, u):
            if u == 1:
                eng.wait_ge(cq[0], 16)
            elif u >= 2:
                eng.wait_ge(cq[u % 2], 16 * (u // 2))
            if u >= NSLOTC:
                eng.wait_ge(pesem, 2 * (u - NSLOTC) + 2)
            eng.dma_start(
                out=cts[u % NSLOTC][:], in_=c_in[u]
            ).then_inc(cq[u % 2], 16)

        def odma(eng, u):
            oz0, noz = UNITS[u][0], UNITS[u][1]
            if u < NU - 1:
                eng.wait_ge(dvsem, 2 * u + 2)  # both copies staged
                dst = out[oz0 : oz0 + noz, :, :, :, :].rearrange(
                    "a b c d e -> (a b) (c d e)"
                )
                eng.dma_start(
                    out=dst, in_=stgs[u % 2][: 16 * noz, :]
                ).then_inc(oss[u % 2], 16)
            else:
                # last unit ships in three pieces, the final one a quarter,
                # so the post-stream tail is one small copy + DMA
                for dv, tt, ha, hb in (
                    (2 * NU - 1, 0, 0, 8), (2 * NU, 1, 0, 4), (2 * NU + 1, 1, 4, 8),
                ):
                    eng.wait_ge(dvsem, dv)
                    dst = out[
                        oz0 : oz0 + noz, :, tt : tt + 1, ha : hb, :
                    ].rearrange("a b c d e -> (a b) (c d e)")
                    eng.dma_start(
                        out=dst,
                        in_=stgs[u % 2][
                            : 16 * noz, 512 * tt + 64 * ha : 512 * tt + 64 * hb
                        ],
                    ).then_inc(oss[u % 2], 16)

        with nc.Block() as block:

            @block.sync
            def _(sp):
                # ALL input DMAs ride the SP ring in exact consumption order
                # (HWDGE rings are FIFO per issuing engine), so ring
                # arbitration can never starve an urgently-needed piece.
                # Pacing: same-sem transfers never overlap, so per-sem
                # counts stay completion-exact; the slot wait keeps a tile
                # resident until its fixup reader (unit u-NSLOT+1) retires.
                def adma(u):
                    if u == 1:
                        sp.wait_ge(xsA[0], 16)
                    elif u >= 2:
                        sp.wait_ge(xsA[u % 2], 16 * (u // 2))
                    if u >= NSLOT:
                        sp.wait_ge(pesem, 2 * (u - NSLOT) + 4)
                    sp.dma_start(
                        out=xts[u % NSLOT][:, 0 : 10 * ROW],
                        in_=x_in[u, :, 0 : 10 * ROW],
                    ).then_inc(xsA[u % 2], 16)

                def bdma(u):
                    if u >= 2:
                        sp.wait_ge(xsB[u % 2], 16 * (u // 2))
                    sp.dma_start(
                        out=xts[u % NSLOT][:, 10 * ROW : 18 * ROW],
                        in_=x_in[u, :, 10 * ROW : 18 * ROW],
                    ).then_inc(xsB[u % 2], 16)

                # ring lead order = first-use order: the 4 tap matrices
                # of group (0,0)'s jy=0 taps, then its yo rows 0..7, then
                # the rest of the 7-oz bands + fixup, then rows 8..9
                sp.dma_start(
                    out=wtile[:, 13 * 128 : 17 * 128],
                    in_=w_in[:, 13 * 128 : 17 * 128],
                ).then_inc(wsem, 16)
                sp.dma_start(
                    out=xts[0][:, 0 : 8 * ROW], in_=x_in[0, :, 0 : 8 * ROW]
                ).then_inc(a0s, 16)
                sp.dma_start(
                    out=wtile[:, 17 * 128 :], in_=w_in[:, 17 * 128 :]
                ).then_inc(ws2, 16)
                sp.dma_start(
                    out=xts[0][:, 8 * ROW : 10 * ROW],
                    in_=x_in[0, :, 8 * ROW : 10 * ROW],
                ).then_inc(xsA[0], 16)
                bdma(0)
                for u in range(1, NU):
                    adma(u)
                    bdma(u)
                # fence the last output DMAs before the NEFF completes;
                # no epilogue sem_clear needed -- the framework preamble
                # dma_resets + clears the whole kernel sem range on every
                # execution, so counts start from zero each run
                sp.wait_ge(oss[0], 16 * ((NU + 1) // 2))
                sp.wait_ge(oss[1], 16 * (NU // 2 + 2))

            @block.scalar
            def _(act):
                # the whole ACT ring is anchored behind the ws2 guard
                # (wt1b on SP done), so it can never starve the critical
                # head; corner tiles 0-2 and wt2 then ride ACT to keep
                # ~1.3MB out of the SP transient, and corner tiles 3-9
                # interleave with output DMAs in wait-clear order so no
                # head-of-line wait can starve a transitively-needed DMA
                act.wait_ge(ws2, 16)
                for u in range(NSLOTC):
                    cdma(act, u)
                act.dma_start(
                    out=wtile[:, : 13 * 128], in_=w_in[:, : 13 * 128]
                ).then_inc(ws2, 16)
                for k in range(NU):
                    if k + NSLOTC < NU:
                        cdma(act, k + NSLOTC)
                    odma(act, k)

            @block.tensor
            def _(pe):
                # Short HAM warm-up on garbage: keeps PE busy while tile 0
                # and wtile stream in, so most real matmuls run at 2.4 GHz.
                # psum bank 7 is discarded by its first start=True.
                for _ in range(WARMUP):
                    pe.matmul(
                        pss[7][:, 0:64], wtile[:, 0:128], wtile[:, 0:64],
                        start=True, stop=True,
                    )
                pe.wait_ge(wsem, 16)
                g = 0
                for u, (oz0, noz, zp0, wkind, fixup) in enumerate(UNITS):
                    if u == 4:
                        pe.wait_ge(ws2, 32)  # 6-oz bands landed
                    for tt in range(2):
                        if u == 0 and tt == 0:
                            pe.wait_ge(a0s, 16)  # yo rows 0..7 (jy=0 taps)
                        else:
                            pe.wait_ge(xsA[u % 2], 16 * (u // 2 + 1))
                        if u + tt == 1:
                            pe.wait_ge(ws2, 16)  # rest of the 7-oz bands
                        if tt == 1:
                            pe.wait_ge(xsB[u % 2], 16 * (u // 2 + 1))
                        if g >= 8:
                            pe.wait_ge(dvsem, g - 7)  # psum bank g%8 evacuated
                        ps = pss[g % 8]
                        if fixup:
                            # (oz0, dz=-2, dy=0, dx=0) from the previous tile
                            x5p = x5s[(u - 1) % NSLOT]
                            pe.matmul(
                                ps[:],
                                wtile[:, 26 * 128 : 27 * 128],
                                x5p[:, 8 * tt + 1 : 8 * tt + 9, 0:1, 0:1, 1:65],
                                start=True, stop=False,
                            )
                        x5 = x5s[u % NSLOT]
                        c5 = c5s[u % NSLOTC]
                        # the very last group runs as two oy-quarter halves
                        # so its evacuation/output pipeline overlaps the
                        # second half's matmuls (shorter kernel tail)
                        halves = (
                            [(0, 4), (4, 8)] if g == 2 * NU - 1 else [(0, 8)]
                        )
                        for ha, hb in halves:
                            nh = hb - ha
                            psh = ps
                            if ha > 0:
                                # second half lands in bank 7 so the first
                                # half can evacuate while it accumulates
                                pe.wait_ge(dvsem, 16)  # bank 7 (g=15) clear
                                psh = pss[7]
                            for j, (dy, dx) in enumerate(REG_TAPS):
                                jy, py = divmod(dy + 2, 2)
                                jx, px = divmod(dx + 2, 2)
                                a0 = 8 * tt + jy + ha
                                rhs = x5[
                                    :, a0 : a0 + nh, py : py + 1, px : px + 1,
                                    jx : jx + 64,
                                ]
                                if g == 0 and ha == 0 and j == 3:
                                    pe.wait_ge(xsA[0], 16)  # yo rows 8..9
                                    pe.wait_ge(ws2, 16)
                                c0 = (13 * wkind + TAPS_XY.index((dy, dx))) * 128
                                pe.matmul(
                                    psh[:, 64 * ha : 64 * hb],
                                    wtile[:, c0 : c0 + 128],
                                    rhs,
                                    start=(j == 0 and not (fixup and ha == 0)),
                                    stop=False,
                                )
                            # merged corner pair {(0,2), (-2,0)}: copy 0 of
                            # the corner tile is the (yp0,px0) phase, copy 1
                            # the same shifted (+1,+1), so one slice serves
                            # both shifts on disjoint K rows
                            if tt == 0 and ha == 0:
                                pe.wait_ge(cq[u % 2], 16 * (u // 2 + 1))
                            a0 = 8 * tt + 1 + ha
                            cm = (27 + wkind) * 128
                            mm = pe.matmul(
                                psh[:, 64 * ha : 64 * hb],
                                wtile[:, cm : cm + 128],
                                c5[:, a0 : a0 + nh, 2:66],
                                start=False,
                                stop=True,
                            )
                            mm.then_inc(pesem, 1)
                        g += 1

            @block.vector
            def _(dve):
                npe = 0
                for g in range(2 * NU):
                    u, tt = divmod(g, 2)
                    M = 16 * UNITS[u][1]
                    if tt == 0 and u >= 2:
                        # stage slot u%2 free: same-parity odmas are serialized,
                        # so the per-parity count is completion-exact
                        dve.wait_ge(oss[u % 2], 16 * (u // 2))
                    halves = [(0, 4), (4, 8)] if g == 2 * NU - 1 else [(0, 8)]
                    for ha, hb in halves:
                        npe += 1
                        dve.wait_ge(pesem, npe)
                        bank = 7 if ha > 0 else g % 8
                        dve.tensor_copy(
                            out=stgs[u % 2][
                                :M, 512 * tt + 64 * ha : 512 * tt + 64 * hb
                            ],
                            in_=pss[bank][:M, 64 * ha : 64 * hb],
                        ).then_inc(dvsem, 1)

    nc.compile()
    return nc


def _get_module():
    global _MODULE
    if _MODULE is None:
        _MODULE = _build_module()
    return _MODULE


def _band_weights(w5):
    """wc[k=(z*8+ic), (13*wkind+j)*128 + ozs*16 + oc] block-banded weights.

    wkind 0: 6-oz window, rel plane = 2*ozs+dzi. wkind 1: 7-oz shifted
    window, rel = 2*ozs+dzi-1 (the z=-1 miss is the fixup's job). Column
    block 26 is the fixup matrix: tap (dz=-2, dy=0, dx=0) for ozs 0 read
    from the previous tile's rel plane 13.
    """
    wc = np.zeros((128, NW, 128), np.float32)
    for j, (dy, dx) in enumerate(TAPS_XY):
        for dzi in range(5):
            dz = dzi - 2
            if dz * dz + dy * dy + dx * dx > 4:
                continue
            blk = w5[:, :, dzi, dy + 2, dx + 2].T  # [ic, oc]
            for ozs in range(6):
                z = 2 * ozs + dzi
                wc[z * 8 : (z + 1) * 8, j, ozs * 16 : ozs * 16 + 16] = blk
            for ozs in range(7):
                z = 2 * ozs + dzi - 1
                if 0 <= z < 16:
                    wc[z * 8 : (z + 1) * 8, 13 + j, ozs * 16 : ozs * 16 + 16] = blk
    wc[13 * 8 : 14 * 8, 26, 0:16] = w5[:, :, 0, 2, 2].T
    # merged corner pair: copy-0 K rows (p < 64) carry tap (0,+2), copy-1
    # rows (p >= 64) tap (-2,0); zi = ozs + 1 (6-oz window) or ozs (7-oz)
    for wkind in range(2):
        col = 27 + wkind
        nozs, zoff = (6, 1) if wkind == 0 else (7, 0)
        for ozs in range(nozs):
            zi = ozs + zoff
            wc[zi * 8 : zi * 8 + 8, col, ozs * 16 : ozs * 16 + 16] = w5[
                :, :, 2, 2, 4
            ].T
            wc[64 + zi * 8 : 64 + zi * 8 + 8, col, ozs * 16 : ozs * 16 + 16] = (
                w5[:, :, 2, 0, 2].T
            )
    return np.ascontiguousarray(wc.reshape(128, NW * 128))


def _shard_core_input(x, b, q):
    """Per-core padded input as NU z-window units [128, 36*132]."""
    xp = np.zeros((IC, ZP, 36, 132), BF16)
    y_lo = 32 * q - 2
    ys_lo, ys_hi = max(0, y_lo), min(128, y_lo + 36)
    xp[:, 2:130, ys_lo - y_lo : ys_hi - y_lo, 2:130] = x[
        b, :, :, ys_lo:ys_hi, :
    ]
    units = np.empty((NU, 128, SUB_FREE), BF16)
    corners = np.zeros((NU, 2, 8, IC, 18, 66), BF16)
    ph00 = xp[:, :, 0::2, 0::2]  # (yp=0, px=0) phase [IC, ZP, 18, 66]
    for i, (_, _, zp0, wkind, _) in enumerate(UNITS):
        u = xp[:, zp0 : zp0 + 16]
        # de-interleave phases: free = (yo 18, yp 2, px 2, xe 66)
        u = u.reshape(IC, 16, 36, 66, 2).transpose(0, 1, 2, 4, 3)
        u = u.reshape(IC, 16, 18, 2, 2, 66)
        units[i] = u.transpose(1, 0, 2, 3, 4, 5).reshape(128, SUB_FREE)
        # corner tile: 8 same-parity planes; copy 0 identity, copy 1
        # shifted (+1,+1) so one rhs slice serves both corner taps
        pl = ph00[:, zp0 + wkind : zp0 + wkind + 16 : 2]  # [IC, 8, 18, 66]
        corners[i, 0] = pl.transpose(1, 0, 2, 3)
        corners[i, 1, :, :, 1:, 1:] = pl.transpose(1, 0, 2, 3)[:, :, :-1, :-1]
    return units, corners.reshape(NU, 128, 18 * 66)


def kernel(x, weight, bias, psi_local):
    global LAST_RESULT
    from concourse.bass_utils import run_bass_kernel_spmd

    x = np.asarray(x, np.float32)
    weight = np.asarray(weight, np.float32)
    bias = np.asarray(bias, np.float32)
    psi_local = np.asarray(psi_local, np.float32)

    w5 = np.einsum("ogk,kzyx->ogzyx", weight, psi_local).astype(np.float32)
    wc = _band_weights(w5).astype(BF16)

    in_maps = []
    for core in range(N_CORES):
        b, q = divmod(core, 4)
        units, corners = _shard_core_input(x, b, q)
        in_maps.append({"xc": units, "cc": corners, "wc": wc})

    nc = _get_module()
    trace = bool(int(os.environ.get("KERNEL_TRACE", "0")))
    res = run_bass_kernel_spmd(
        nc, in_maps, core_ids=list(range(N_CORES)), trace=trace
    )
    LAST_RESULT = res

    out = np.empty((2, OC, 64, 64, 64), np.float32)
    for core in range(N_CORES):
        b, q = divmod(core, 4)
        co = res.results[core]["out"].astype(np.float32).reshape(64, 16, 16, 64)
        out[b, :, :, 16 * q : 16 * q + 16, :] = co.transpose(1, 0, 2, 3)
    out += bias[None, :, None, None, None]
    return out


# revision 20
# speedup vs baseline: 1.0198x; 1.0198x over previous
"""Trainium2 Bass kernel for EquidistantDiscreteContinuousConv3d.

Math: out = conv3d(x, einsum('ogk,kzyx->ogzyx', weight, psi_local), stride 2,
pad 2) + bias, with x [2,8,128,128,128] -> out [2,16,64,64,64].

The dense 5^3 kernel only has taps within Euclidean radius 2 (33 of 125
offsets are nonzero). Sharding: 8 cores = batch(2) x y-quarters(4); each core
computes out[b, :, :, 16q:16q+16] from an overlapping, zero-padded input
slab. No collectives -- halos materialize as overlapping host-side slices.

Device mapping: the tensor engine contracts K = (z_window(16) x ic(8)) = 128
partitions, with M = (oz_sub x oc(16)) packed into a block-banded weight
matrix (band encodes the dz taps), looped over the 13 (dy, dx) stencil taps
that accumulate in PSUM. Giving each core the FULL z extent (64 oz) makes
the 6-oz-per-16-plane-window blocking waste only boundary slots. Four lead
units cover 7 oz each via a shifted window [2*oz0-1, +16) plus a one-matmul
"fixup" (the single missing (oz0, dz=-2, dy=0, dx=0) combo) that contracts
8 K-rows of the PREVIOUS unit's tile; the remaining six units are plain
6-oz windows. 10 units x 2 oy-halves = 20 PSUM groups, 266 matmuls of
N=512 vs 312 for a uniform-6 z-sharded layout.

rhs slices come from a phase-decomposed (even/odd y and x, de-interleaved so
the innermost 64 x-positions are contiguous) view of the input tile. Each
unit arrives as two parallel half-DMAs (yo rows 0..9 on the SP queue, rows
10..17 on the ACT queue) so tile 0 lands fast; wtile and the 10 output DMAs
ride the otherwise-idle GPSIMD queue. Output is written bf16 (cast on the
psum->stage copy) and widened to f32 on the host.

Raw Bacc pipeline per core (static, fully unrolled; no TileContext):
  SP    : 10 paced input A-half DMAs + even-unit output DMAs, end sem clear
  ACT   : wtile DMA, 10 paced input B-half DMAs + odd-unit output DMAs
  PE    : short HAM warm-up, then 20 groups x (13|14) banded matmuls
  DVE   : 20 psum->stage bf16 copies (stage slot = unit%2)
"""

import os

import ml_dtypes
import numpy as np

BF16 = ml_dtypes.bfloat16

IC, OC = 8, 16
TAPS_XY = [
    (dy, dx) for dy in range(-2, 3) for dx in range(-2, 3) if dy * dy + dx * dx <= 4
]  # 13 taps
SUB_FREE = 36 * 132  # per-unit free size: (yo 18, yp 2, px 2, xe 66)
ROW = 2 * 2 * 66  # one yo row = (yp, px, xe) block of 264 elements
N_CORES = 8
NSLOT = 6
WARMUP = 112

# units: (oz0, noz, zp0, wkind, fixup) -- zp = z + 2 (host pad), window is
# zp [zp0, zp0+16). wkind 0: band rel plane = 2*ozs+dzi (6-oz); wkind 1:
# rel = 2*ozs+dzi-1 (7-oz, shifted window).
UNITS = [(7 * u, 7, 14 * u + 1, 1, u > 0) for u in range(4)] + [
    (28 + 6 * k, 6, 56 + 12 * k, 0, False) for k in range(6)
]
NU = len(UNITS)
NW = 29  # 13 (6-oz band) + 13 (7-oz) + fixup + 2 merged-corner matrices
MERGED = ((0, 2), (-2, 0))  # corner-tap pair computed by one matmul
REG_TAPS = [t for t in TAPS_XY if t not in MERGED]  # 11 regular taps
CFREE = 18 * 66  # corner tile free size (ay 18, xv 66)
NSLOTC = 3
ZP = 132  # padded z planes on host (z = zp - 2)

_MODULE = None
LAST_RESULT = None  # BassKernelResults of the most recent run (for test harness)


def _build_module():
    from contextlib import ExitStack

    import concourse.bacc as bacc
    import concourse.mybir as mybir

    f32 = mybir.dt.float32
    bf16 = mybir.dt.bfloat16

    nc = bacc.Bacc()
    x_in = nc.dram_tensor("xc", [NU, 128, SUB_FREE], bf16, kind="ExternalInput")
    c_in = nc.dram_tensor("cc", [NU, 128, CFREE], bf16, kind="ExternalInput")
    w_in = nc.dram_tensor("wc", [128, NW * 128], bf16, kind="ExternalInput")
    out = nc.dram_tensor("out", [64, 16, 2, 8, 64], bf16, kind="ExternalOutput")

    with ExitStack() as ctx:
        wsem = ctx.enter_context(nc.semaphore("wsem"))
        ws2 = ctx.enter_context(nc.semaphore("ws2"))
        a0s = ctx.enter_context(nc.semaphore("a0s"))
        cq = [ctx.enter_context(nc.semaphore(f"cq{i}")) for i in range(2)]
        xsA = [ctx.enter_context(nc.semaphore(f"xsemA{i}")) for i in range(2)]
        xsB = [ctx.enter_context(nc.semaphore(f"xsemB{i}")) for i in range(2)]
        pesem = ctx.enter_context(nc.semaphore("pesem"))
        dvsem = ctx.enter_context(nc.semaphore("dvsem"))
        oss = [ctx.enter_context(nc.semaphore(f"osem{i}")) for i in range(2)]
        wtile = ctx.enter_context(nc.sbuf_tensor("wtile", [128, NW * 128], bf16))
        xts = [
            ctx.enter_context(nc.sbuf_tensor(f"xt{i}", [128, SUB_FREE], bf16))
            for i in range(NSLOT)
        ]
        cts = [
            ctx.enter_context(nc.sbuf_tensor(f"ct{i}", [128, CFREE], bf16))
            for i in range(NSLOTC)
        ]
        stgs = [
            ctx.enter_context(nc.sbuf_tensor(f"stg{i}", [128, 2 * 512], bf16))
            for i in range(2)
        ]
        pss = [
            ctx.enter_context(nc.psum_tensor(f"ps{i}", [128, 512], f32))
            for i in range(8)
        ]
        x5s = [
            t[:].rearrange("p (a b d c) -> p a b d c", a=18, b=2, d=2, c=66)
            for t in xts
        ]
        c5s = [
            t[:].rearrange("p (a c) -> p a c", a=18, c=66) for t in cts
        ]

        def cdma(eng, u):
            if u == 1:
                eng.wait_ge(cq[0], 16)
            elif u >= 2:
                eng.wait_ge(cq[u % 2], 16 * (u // 2))
            if u >= NSLOTC:
                eng.wait_ge(pesem, 2 * (u - NSLOTC) + 2)
            eng.dma_start(
                out=cts[u % NSLOTC][:], in_=c_in[u]
            ).then_inc(cq[u % 2], 16)

        def odma(eng, u):
            oz0, noz = UNITS[u][0], UNITS[u][1]
            if u < NU - 1:
                eng.wait_ge(dvsem, 2 * u + 2)  # both copies staged
                dst = out[oz0 : oz0 + noz, :, :, :, :].rearrange(
                    "a b c d e -> (a b) (c d e)"
                )
                eng.dma_start(
                    out=dst, in_=stgs[u % 2][: 16 * noz, :]
                ).then_inc(oss[u % 2], 16)
            else:
                # last unit ships in three pieces, the final one a quarter,
                # so the post-stream tail is one small copy + DMA
                for dv, tt, ha, hb in (
                    (2 * NU - 1, 0, 0, 8), (2 * NU, 1, 0, 4), (2 * NU + 1, 1, 4, 8),
                ):
                    eng.wait_ge(dvsem, dv)
                    dst = out[
                        oz0 : oz0 + noz, :, tt : tt + 1, ha : hb, :
                    ].rearrange("a b c d e -> (a b) (c d e)")
                    eng.dma_start(
                        out=dst,
                        in_=stgs[u % 2][
                            : 16 * noz, 512 * tt + 64 * ha : 512 * tt + 64 * hb
                        ],
                    ).then_inc(oss[u % 2], 16)

        with nc.Block() as block:

            @block.sync
            def _(sp):
                # ALL input DMAs ride the SP ring in exact consumption order
                # (HWDGE rings are FIFO per issuing engine), so ring
                # arbitration can never starve an urgently-needed piece.
                # Pacing: same-sem transfers never overlap, so per-sem
                # counts stay completion-exact; the slot wait keeps a tile
                # resident until its fixup reader (unit u-NSLOT+1) retires.
                def adma(u):
                    if u == 1:
                        sp.wait_ge(xsA[0], 16)
                    elif u >= 2:
                        sp.wait_ge(xsA[u % 2], 16 * (u // 2))
                    if u >= NSLOT:
                        sp.wait_ge(pesem, 2 * (u - NSLOT) + 4)
                    sp.dma_start(
                        out=xts[u % NSLOT][:, 0 : 10 * ROW],
                        in_=x_in[u, :, 0 : 10 * ROW],
                    ).then_inc(xsA[u % 2], 16)

                def bdma(u):
                    if u >= 2:
                        sp.wait_ge(xsB[u % 2], 16 * (u // 2))
                    sp.dma_start(
                        out=xts[u % NSLOT][:, 10 * ROW : 18 * ROW],
                        in_=x_in[u, :, 10 * ROW : 18 * ROW],
                    ).then_inc(xsB[u % 2], 16)

                # ring lead order = first-use order: the 4 tap matrices
                # of group (0,0)'s jy=0 taps, then its yo rows 0..7, then
                # the rest of the 7-oz bands + fixup, then rows 8..9
                sp.dma_start(
                    out=wtile[:, 13 * 128 : 17 * 128],
                    in_=w_in[:, 13 * 128 : 17 * 128],
                ).then_inc(wsem, 16)
                sp.dma_start(
                    out=xts[0][:, 0 : 8 * ROW], in_=x_in[0, :, 0 : 8 * ROW]
                ).then_inc(a0s, 16)
                sp.dma_start(
                    out=wtile[:, 17 * 128 :], in_=w_in[:, 17 * 128 :]
                ).then_inc(ws2, 16)
                sp.dma_start(
                    out=xts[0][:, 8 * ROW : 10 * ROW],
                    in_=x_in[0, :, 8 * ROW : 10 * ROW],
                ).then_inc(xsA[0], 16)
                cdma(sp, 0)  # corner tile 0: group (0,0)'s 12th matmul
                bdma(0)
                for u in range(1, NU):
                    adma(u)
                    if u <= 2:  # corner tiles 1-2 stay on the stable ring
                        cdma(sp, u)
                    bdma(u)
                # fence the last output DMAs before the NEFF completes;
                # no epilogue sem_clear needed -- the framework preamble
                # dma_resets + clears the whole kernel sem range on every
                # execution, so counts start from zero each run
                sp.wait_ge(oss[0], 16 * ((NU + 1) // 2))
                sp.wait_ge(oss[1], 16 * (NU // 2 + 2))

            @block.scalar
            def _(act):
                # wt2 (6-oz bands, needed from unit 4) rides ACT so it
                # stays out of the SP ring's DMA-limited transient; the
                # ws2 wait orders it provably after wt1b (same semaphore).
                # Corner tiles 3-9 interleave with output DMAs in
                # wait-clear order so no head-of-line wait can starve a
                # transitively-needed DMA
                act.wait_ge(ws2, 16)
                act.dma_start(
                    out=wtile[:, : 13 * 128], in_=w_in[:, : 13 * 128]
                ).then_inc(ws2, 16)
                for k in range(NU):
                    if k + NSLOTC < NU:
                        cdma(act, k + NSLOTC)
                    odma(act, k)

            @block.tensor
            def _(pe):
                # Short HAM warm-up on garbage: keeps PE busy while tile 0
                # and wtile stream in, so most real matmuls run at 2.4 GHz.
                # psum bank 7 is discarded by its first start=True.
                for _ in range(WARMUP):
                    pe.matmul(
                        pss[7][:, 0:64], wtile[:, 0:128], wtile[:, 0:64],
                        start=True, stop=True,
                    )
                pe.wait_ge(wsem, 16)
                g = 0
                for u, (oz0, noz, zp0, wkind, fixup) in enumerate(UNITS):
                    if u == 4:
                        pe.wait_ge(ws2, 32)  # 6-oz bands landed
                    for tt in range(2):
                        if u == 0 and tt == 0:
                            pe.wait_ge(a0s, 16)  # yo rows 0..7 (jy=0 taps)
                        else:
                            pe.wait_ge(xsA[u % 2], 16 * (u // 2 + 1))
                        if u + tt == 1:
                            pe.wait_ge(ws2, 16)  # rest of the 7-oz bands
                        if tt == 1:
                            pe.wait_ge(xsB[u % 2], 16 * (u // 2 + 1))
                        if g >= 8:
                            pe.wait_ge(dvsem, g - 7)  # psum bank g%8 evacuated
                        ps = pss[g % 8]
                        if fixup:
                            # (oz0, dz=-2, dy=0, dx=0) from the previous tile
                            x5p = x5s[(u - 1) % NSLOT]
                            pe.matmul(
                                ps[:],
                                wtile[:, 26 * 128 : 27 * 128],
                                x5p[:, 8 * tt + 1 : 8 * tt + 9, 0:1, 0:1, 1:65],
                                start=True, stop=False,
                            )
                        x5 = x5s[u % NSLOT]
                        c5 = c5s[u % NSLOTC]
                        # the very last group runs as two oy-quarter halves
                        # so its evacuation/output pipeline overlaps the
                        # second half's matmuls (shorter kernel tail)
                        halves = (
                            [(0, 4), (4, 8)] if g == 2 * NU - 1 else [(0, 8)]
                        )
                        for ha, hb in halves:
                            nh = hb - ha
                            psh = ps
                            if ha > 0:
                                # second half lands in bank 7 so the first
                                # half can evacuate while it accumulates
                                pe.wait_ge(dvsem, 16)  # bank 7 (g=15) clear
                                psh = pss[7]
                            for j, (dy, dx) in enumerate(REG_TAPS):
                                jy, py = divmod(dy + 2, 2)
                                jx, px = divmod(dx + 2, 2)
                                a0 = 8 * tt + jy + ha
                                rhs = x5[
                                    :, a0 : a0 + nh, py : py + 1, px : px + 1,
                                    jx : jx + 64,
                                ]
                                if g == 0 and ha == 0 and j == 3:
                                    pe.wait_ge(xsA[0], 16)  # yo rows 8..9
                                    pe.wait_ge(ws2, 16)
                                c0 = (13 * wkind + TAPS_XY.index((dy, dx))) * 128
                                pe.matmul(
                                    psh[:, 64 * ha : 64 * hb],
                                    wtile[:, c0 : c0 + 128],
                                    rhs,
                                    start=(j == 0 and not (fixup and ha == 0)),
                                    stop=False,
                                )
                            # merged corner pair {(0,2), (-2,0)}: copy 0 of
                            # the corner tile is the (yp0,px0) phase, copy 1
                            # the same shifted (+1,+1), so one slice serves
                            # both shifts on disjoint K rows
                            if tt == 0 and ha == 0:
                                pe.wait_ge(cq[u % 2], 16 * (u // 2 + 1))
                            a0 = 8 * tt + 1 + ha
                            cm = (27 + wkind) * 128
                            mm = pe.matmul(
                                psh[:, 64 * ha : 64 * hb],
                                wtile[:, cm : cm + 128],
                                c5[:, a0 : a0 + nh, 2:66],
                                start=False,
                                stop=True,
                            )
                            mm.then_inc(pesem, 1)
                        g += 1

            @block.vector
            def _(dve):
                npe = 0
                for g in range(2 * NU):
                    u, tt = divmod(g, 2)
                    M = 16 * UNITS[u][1]
                    if tt == 0 and u >= 2:
                        # stage slot u%2 free: same-parity odmas are serialized,
                        # so the per-parity count is completion-exact
                        dve.wait_ge(oss[u % 2], 16 * (u // 2))
                    halves = [(0, 4), (4, 8)] if g == 2 * NU - 1 else [(0, 8)]
                    for ha, hb in halves:
                        npe += 1
                        dve.wait_ge(pesem, npe)
                        bank = 7 if ha > 0 else g % 8
                        dve.tensor_copy(
                            out=stgs[u % 2][
                                :M, 512 * tt + 64 * ha : 512 * tt + 64 * hb
                            ],
                            in_=pss[bank][:M, 64 * ha : 64 * hb],
                        ).then_inc(dvsem, 1)

    nc.compile()
    return nc


def _get_module():
    global _MODULE
    if _MODULE is None:
        _MODULE = _build_module()
    return _MODULE


def _band_weights(w5):
    """wc[k=(z*8+ic), (13*wkind+j)*128 + ozs*16 + oc] block-banded weights.

    wkind 0: 6-oz window, rel plane = 2*ozs+dzi. wkind 1: 7-oz shifted
    window, rel = 2*ozs+dzi-1 (the z=-1 miss is the fixup's job). Column
    block 26 is the fixup matrix: tap (dz=-2, dy=0, dx=0) for ozs 0 read
    from the previous tile's rel plane 13.
    """
    wc = np.zeros((128, NW, 128), np.float32)
    for j, (dy, dx) in enumerate(TAPS_XY):
        for dzi in range(5):
            dz = dzi - 2
            if dz * dz + dy * dy + dx * dx > 4:
                continue
            blk = w5[:, :, dzi, dy + 2, dx + 2].T  # [ic, oc]
            for ozs in range(6):
                z = 2 * ozs + dzi
                wc[z * 8 : (z + 1) * 8, j, ozs * 16 : ozs * 16 + 16] = blk
            for ozs in range(7):
                z = 2 * ozs + dzi - 1
                if 0 <= z < 16:
                    wc[z * 8 : (z + 1) * 8, 13 + j, ozs * 16 : ozs * 16 + 16] = blk
    wc[13 * 8 : 14 * 8, 26, 0:16] = w5[:, :, 0, 2, 2].T
    # merged corner pair: copy-0 K rows (p < 64) carry tap (0,+2), copy-1
    # rows (p >= 64) tap (-2,0); zi = ozs + 1 (6-oz window) or ozs (7-oz)
    for wkind in range(2):
        col = 27 + wkind
        nozs, zoff = (6, 1) if wkind == 0 else (7, 0)
        for ozs in range(nozs):
            zi = ozs + zoff
            wc[zi * 8 : zi * 8 + 8, col, ozs * 16 : ozs * 16 + 16] = w5[
                :, :, 2, 2, 4
            ].T
            wc[64 + zi * 8 : 64 + zi * 8 + 8, col, ozs * 16 : ozs * 16 + 16] = (
                w5[:, :, 2, 0, 2].T
            )
    return np.ascontiguousarray(wc.reshape(128, NW * 128))


def _shard_core_input(x, b, q):
    """Per-core padded input as NU z-window units [128, 36*132]."""
    xp = np.zeros((IC, ZP, 36, 132), BF16)
    y_lo = 32 * q - 2
    ys_lo, ys_hi = max(0, y_lo), min(128, y_lo + 36)
    xp[:, 2:130, ys_lo - y_lo : ys_hi - y_lo, 2:130] = x[
        b, :, :, ys_lo:ys_hi, :
    ]
    units = np.empty((NU, 128, SUB_FREE), BF16)
    corners = np.zeros((NU, 2, 8, IC, 18, 66), BF16)
    ph00 = xp[:, :, 0::2, 0::2]  # (yp=0, px=0) phase [IC, ZP, 18, 66]
    for i, (_, _, zp0, wkind, _) in enumerate(UNITS):
        u = xp[:, zp0 : zp0 + 16]
        # de-interleave phases: free = (yo 18, yp 2, px 2, xe 66)
        u = u.reshape(IC, 16, 36, 66, 2).transpose(0, 1, 2, 4, 3)
        u = u.reshape(IC, 16, 18, 2, 2, 66)
        units[i] = u.transpose(1, 0, 2, 3, 4, 5).reshape(128, SUB_FREE)
        # corner tile: 8 same-parity planes; copy 0 identity, copy 1
        # shifted (+1,+1) so one rhs slice serves both corner taps
        pl = ph00[:, zp0 + wkind : zp0 + wkind + 16 : 2]  # [IC, 8, 18, 66]
        corners[i, 0] = pl.transpose(1, 0, 2, 3)
        corners[i, 1, :, :, 1:, 1:] = pl.transpose(1, 0, 2, 3)[:, :, :-1, :-1]
    return units, corners.reshape(NU, 128, 18 * 66)


def kernel(x, weight, bias, psi_local):
    global LAST_RESULT
    from concourse.bass_utils import run_bass_kernel_spmd

    x = np.asarray(x, np.float32)
    weight = np.asarray(weight, np.float32)
    bias = np.asarray(bias, np.float32)
    psi_local = np.asarray(psi_local, np.float32)

    w5 = np.einsum("ogk,kzyx->ogzyx", weight, psi_local).astype(np.float32)
    wc = _band_weights(w5).astype(BF16)

    in_maps = []
    for core in range(N_CORES):
        b, q = divmod(core, 4)
        units, corners = _shard_core_input(x, b, q)
        in_maps.append({"xc": units, "cc": corners, "wc": wc})

    nc = _get_module()
    trace = bool(int(os.environ.get("KERNEL_TRACE", "0")))
    res = run_bass_kernel_spmd(
        nc, in_maps, core_ids=list(range(N_CORES)), trace=trace
    )
    LAST_RESULT = res

    out = np.empty((2, OC, 64, 64, 64), np.float32)
    for core in range(N_CORES):
        b, q = divmod(core, 4)
        co = res.results[core]["out"].astype(np.float32).reshape(64, 16, 16, 64)
        out[b, :, :, 16 * q : 16 * q + 16, :] = co.transpose(1, 0, 2, 3)
    out += bias[None, :, None, None, None]
    return out
